# revision 68
# baseline (speedup 1.0000x reference)
"""SSIM-pyramid loss kernel for 8 Trainium2 NeuronCores (Bass/Tile).

Math: the reference loss per pyramid level reduces EXACTLY (to ~1e-8 rel) to
    loss_l = 2 - 2*mean(sig12 / (sqrt(sig1+eps)*sqrt(sig2+eps)))
because sum_k n1^2 over a window = 121*sig1/s1^2 ~= 121*(1 - O(eps/sig)),
with eps=1e-10 and sig >= 3e-3 for these inputs.  So per level we need only
5 box-filtered maps: box(x1), box(x2), box(x1^2), box(x2^2), box(x1*x2).

Distribution: batch b = core//4, row-band i = core%4 (128 rows of L0 per
core).  Each core computes its band of all 4 pyramid levels from a padded
222-row slice of the level-0 images, using per-core banded matrices (inputs)
that encode box-filter truncation and bicubic edge clamping.  Per-core
partial sums are combined on the host.

Box filters / downsamples run on the TensorEngine as banded matmuls; every
vertical pass uses stationary=data-chunk matmuls that emit the transposed
intermediate directly in PSUM (no DMA/xbar transposes anywhere); at levels
1/2 the fused av|dv mobile makes one V pass feed both box and downsample.
Pointwise math uses fused DVE ops (scalar_tensor_tensor with accum_out) and
a single Abs_reciprocal_sqrt activation; its table set is pre-warmed during
the input DMAs.  Inputs are packed into 128-row tiles so every DMA spreads
across all 16 DMA engines.
"""

import sys

sys.path.insert(0, "/opt/trn_rl_repo")

import numpy as np
import ml_dtypes

import concourse.bass as bass  # noqa: E402
import concourse.mybir as mybir  # noqa: E402
import concourse.tile as tile  # noqa: E402
from concourse import bacc  # noqa: E402
from concourse.bass_utils import run_bass_kernel_spmd  # noqa: E402

F32 = mybir.dt.float32
BF16 = mybir.dt.bfloat16
FP16 = mybir.dt.float16

# Whole PE path runs fp16: fp32 matmuls cost 2x (HI/LO passes), bf16 loses
# ~2e-3 accuracy through the sig cancellations, fp16 loses only ~1.6e-4 and
# enables the 2-byte DMA xbar transpose.  PSUM accumulation and the pointwise
# chain stay fp32.
DT_BOX = FP16
NP_BOX = np.float16

WS, PAD = 11, 5
BIC = np.array([-0.09375, 0.59375, 0.59375, -0.09375], np.float64)
PYR_W = (0.2, 0.4, 0.6, 0.8)
NLVL = 4
H = [512, 256, 128, 64]  # = W per level
R = [128, 64, 32, 16]  # band rows per core per level
AluOp = mybir.AluOpType
ActFn = mybir.ActivationFunctionType


# ----------------------------------------------------------------------------
# geometry
# ----------------------------------------------------------------------------
def _lr_ranges(i):
    """Row ranges (unclamped, fixed size) each core carries per level."""
    lr = [None] * NLVL
    s3 = 16 * i
    lr[3] = (s3 - PAD, s3 + 16 + PAD)
    for l in (2, 1, 0):
        s = R[l] * i
        box = (s - PAD, s + R[l] + PAD)
        a1, b1 = lr[l + 1]
        ds = (2 * a1 - 1, 2 * (b1 - 1) + 2 + 1)  # taps 2j-1..2j+2 for j in lr[l+1]
        lr[l] = (min(box[0], ds[0]), max(box[1], ds[1]))
    return lr


NK = [222, 110, 54, 26]  # sizes of lr ranges (identical for all cores)
for _i in range(4):
    _lr = _lr_ranges(_i)
    assert [b - a for a, b in _lr] == NK, (_i, _lr)

# horizontal box-filter chunking: aligned 128 chunks, chunk0 streams full W
# (chunk0's full-width write also resets the PSUM accumulation group); each
# level's pointwise reads only the lanes its own box_h wrote, so no extra
# init streaming is needed
def _bh_windows(W_, full0=None):
    ch = []
    ncw = max(1, W_ // 128)
    for j in range(ncw):
        if j == 0:
            ch.append((0, 0, full0 or W_))
        else:
            ch.append((j, 128 * j - PAD, min(W_, 128 * j + 128 + PAD)))
    return ch


def _dh_windows(W_):  # per-region in-chunks for stride-2 4-tap downsample
    Wn = W_ // 2
    ch = []
    ncw = max(1, W_ // 128)
    for j in range(ncw):
        if j == 0:
            ch.append((0, 0, Wn))
        else:
            ch.append((j, 64 * j - 1, min(Wn, 64 * j + 65)))
    return ch


BH_CH = [_bh_windows(H[l]) for l in range(NLVL)]
DH_CH = [_dh_windows(H[l]) for l in range(NLVL - 1)]
BH_OFF, _o = [], 0
for l in range(NLVL):
    offs = []
    for (_, lo, hi) in BH_CH[l]:
        offs.append(_o)
        _o += hi - lo
    BH_OFF.append(offs)
BH_COLS = _o
DH_OFF, _o = [], 0
for l in range(NLVL - 1):
    offs = []
    for (_, lo, hi) in DH_CH[l]:
        offs.append(_o)
        _o += hi - lo
    DH_OFF.append(offs)
DH_COLS = _o

NKP = [None, 112, 64, 32]  # Dv output rows padded to a multiple of 16
# fused constant layout: [av0 | av1 dv1 | av2 dv2 | av3 | dv0]; av_l and dv_l
# are adjacent for l=1,2 so one V matmul serves box and downsample
AV_OFF = [0, 128, 256, 320]
DV_OFF = [336, 192, 288]
AVD_COLS = 448


# ----------------------------------------------------------------------------
# host-side per-core constant matrices
# ----------------------------------------------------------------------------
def _build_core_mats(i):
    lr = _lr_ranges(i)

    avd = np.zeros((NK[0], AVD_COLS), np.float64)
    for l in range(NLVL):
        a, _b = lr[l]
        s = R[l] * i
        for k in range(NK[l]):
            g = a + k
            for m in range(R[l]):
                if abs(g - (s + m)) <= PAD:
                    avd[k, AV_OFF[l] + m] = 1.0

    for l in range(NLVL - 1):
        a, _b = lr[l]
        an, bn = lr[l + 1]
        for m in range(NK[l + 1]):
            j = an + m
            if j < 0 or j >= H[l + 1]:
                continue
            for t in range(4):
                src = min(max(2 * j - 1 + t, 0), H[l] - 1)
                k = src - a
                assert 0 <= k < NK[l], (l, i, j, src)
                avd[k, DV_OFF[l] + m] += BIC[t]

    return avd.astype(NP_BOX)  # taps 1.0 / BIC exact in fp16


def _build_shared_mats():
    bhm = np.zeros((128, BH_COLS), np.float64)
    for l in range(NLVL):
        for (j, lo, hi), off in zip(BH_CH[l], BH_OFF[l]):
            base = 128 * j
            for p in range(min(128, H[l] - base)):
                w = base + p
                for wp in range(lo, hi):
                    if abs(w - wp) <= PAD:
                        bhm[p, off + (wp - lo)] = 1.0

    dhm = np.zeros((128, DH_COLS), np.float64)
    for l in range(NLVL - 1):
        for (j, lo, hi), off in zip(DH_CH[l], DH_OFF[l]):
            base = 128 * j
            for wp in range(lo, hi):
                for t in range(4):
                    src = min(max(2 * wp - 1 + t, 0), H[l] - 1)
                    p = src - base
                    if 0 <= p < 128:
                        dhm[p, off + (wp - lo)] += BIC[t]
    return bhm.astype(NP_BOX), dhm.astype(NP_BOX)


def _band_slices(img1, img2, b, i):
    """[128, 2048] packed band: rows 0:128 in cols 0:1024, rows 128:222 in
    cols 1024:2048 (so the DMA spreads across all 16 engines)."""
    a, e = _lr_ranges(i)[0]
    band = np.zeros((NK[0], 1024), np.float32)
    lo, hi = max(a, 0), min(e, 512)
    band[lo - a : hi - a, 0:512] = img1[b, 0, lo:hi, :]
    band[lo - a : hi - a, 512:1024] = img2[b, 0, lo:hi, :]
    out = np.zeros((128, 2048), np.float32)
    out[0:128, 0:1024] = band[0:128]
    out[0 : NK[0] - 128, 1024:2048] = band[128:]
    return out.astype(NP_BOX)


def _pack_band_rows(m):
    """[222, C] -> [128, 2C]: rows 128:222 packed into the right half."""
    c = m.shape[1]
    out = np.zeros((128, 2 * c), m.dtype)
    out[0:128, 0:c] = m[0:128]
    out[0 : m.shape[0] - 128, c:] = m[128:]
    return out


# ----------------------------------------------------------------------------
# device program
# ----------------------------------------------------------------------------
BHD_COLS = BH_COLS + DH_COLS  # bh | dh fused side by side


def build_program():
    nc = bacc.Bacc("TRN2", target_bir_lowering=False)

    # band rows 128:222 are packed into the right half of the 128-partition
    # tiles: DMAs with >=128 rows of >=1.7KB spread over all 16 DMA engines,
    # while a 94-row DMA lands on a single engine (~10x slower)
    ximg = nc.dram_tensor("ximg", [128, 2048], DT_BOX, kind="ExternalInput")
    avdm = nc.dram_tensor("avdm", [128, 2 * AVD_COLS], DT_BOX, kind="ExternalInput")
    bhdm = nc.dram_tensor("bhdm", [128, BHD_COLS], DT_BOX, kind="ExternalInput")
    outp = nc.dram_tensor("out", [128, 4], F32, kind="ExternalOutput")

    with tile.TileContext(nc) as tc:
        with (
            tc.tile_pool(name="sb1", bufs=1) as sb1,
            tc.tile_pool(name="sb2", bufs=2) as sb2,
            tc.tile_pool(name="ps_box", bufs=5, space="PSUM") as ps_box,
            tc.tile_pool(name="ps_work", bufs=3, space="PSUM") as ps_work,
        ):
            _emit(nc, tc, sb1, sb2, ps_box, ps_work, ximg, avdm, bhdm, outp)

    nc.compile()
    return nc


def _emit(nc, tc, sb1, sb2, ps_box, ps_work, ximg, avdm, bhdm, outp):
    # ---- load constants & input band -------------------------------------
    avd = sb1.tile([128, 2 * AVD_COLS], DT_BOX, tag="avd")
    bhd = sb1.tile([128, BHD_COLS], DT_BOX, tag="bhd")
    xt0 = sb1.tile([128, 2048], DT_BOX, tag="xt0")
    # >=128-row DMAs; each spreads over all 16 DMA engines.  Left band half
    # and avd first (they gate the first V matmuls); bhd (needed last, by the
    # H passes) queues behind both ximg halves
    nc.sync.dma_start(xt0[:, 0:1024], ximg[:, 0:1024])
    nc.scalar.dma_start(avd[:], avdm[:])
    nc.sync.dma_start(xt0[0:64, 1024:2048], ximg[0:64, 1024:2048])
    nc.scalar.dma_start(xt0[64:128, 1024:2048], ximg[64:128, 1024:2048])
    nc.sync.dma_start(bhd[:], bhdm[:])

    def av_ap(kidx, kk, c0, c1):
        return avd[0:kk, kidx * AVD_COLS + c0 : kidx * AVD_COLS + c1]

    def dv_ap(kidx, kk, c0, c1):
        return avd[0:kk, kidx * AVD_COLS + c0 : kidx * AVD_COLS + c1]

    def bh_ap(p, c0, c1):
        return bhd[0:p, c0:c1]

    def dh_ap(p, c0, c1):
        return bhd[0:p, BH_COLS + c0 : BH_COLS + c1]

    acc = sb1.tile([128, 4], F32, tag="acc")
    nc.vector.memset(acc[:], 0.0)

    # warm the abs_reciprocal_sqrt ACT table set (it also contains Square and
    # Copy) while input DMAs stream, so no 1.3us table reload fires mid-kernel
    warm = sb1.tile([128, 1], F32, tag="warm")
    nc.vector.memset(warm[:], 1.0)
    nc.scalar.activation(warm[:], warm[:], ActFn.Abs_reciprocal_sqrt)

    # per-level x tiles (levels 1..3 produced on-chip); level 0 is packed
    # [128, 2048] with band rows 128:222 in columns 1024:2048
    xt = [
        xt0,
        sb1.tile([NK[1], 512], DT_BOX, tag="xt1", name="xt1"),
        sb1.tile([NK[2], 256], DT_BOX, tag="xt2", name="xt2"),
        sb1.tile([NK[3], 128], DT_BOX, tag="xt3", name="xt3"),
    ]
    # (rows, col_base) sub-bands of each level's tile
    KT = [
        [(128, 0), (NK[0] - 128, 1024)],
        [(NK[1], 0)],
        [(NK[2], 0)],
        [(NK[3], 0)],
    ]

    # deep-level box maps parked in PSUM: map -> [128, 448] tile
    # L1 at [0:64,0:256], L2 at [0:32,256:384], L3 at [0:16,384:448]
    deep_off = {1: 0, 2: 256, 3: 384}
    deep_w = {1: 256, 2: 128, 3: 64}
    box_deep = None

    copy_rr = [0]

    def copy_cast(dst_ap, src_ap):
        # PSUM->SBUF copies alternate between DVE and ACT
        if copy_rr[0] % 2 == 0:
            nc.vector.tensor_copy(dst_ap, src_ap)
        else:
            nc.scalar.activation(dst_ap, src_ap, ActFn.Copy)
        copy_rr[0] += 1

    def box_v(l, groups, fuse_ds=False, xn_ps=None):
        """z-maps + transposed-output vertical pass for level l.  groups:
        tuples of map indices sharing one PSUM tile and one copy_cast.
        With fuse_ds the x maps' mobile is the fused av|dv block, producing
        box-V and downsample-V in one matmul.  Returns per-map (vT AP,
        chunk stride)."""
        Wl, Rl, nk = H[l], R[l], NK[l]
        t = xt[l]
        ktiles = KT[l]

        # z-maps: zz = x*x unscaled (the x121 is folded into the pointwise
        # stt ops).  Deep levels square the downsample PSUM directly on ACT,
        # overlapping the xt copy instead of waiting for it.
        zz_t, z12_t = [], []
        for (kk, cb) in ktiles:
            zz = sb2.tile([kk, 2 * Wl], DT_BOX, tag=f"zz{len(zz_t)}", name="zz")
            z12 = sb2.tile([kk, Wl], DT_BOX, tag=f"z12{len(z12_t)}", name="z12")
            if xn_ps is not None:
                nc.scalar.activation(
                    zz[:], xn_ps[0:kk, 0 : 2 * Wl], ActFn.Square
                )
            elif cb == 0:
                # ktile-a zz on ACT (idle while the band streams in)
                nc.scalar.activation(
                    zz[:], t[0:kk, cb : cb + 2 * Wl], ActFn.Square
                )
            else:
                # ktile-b zz on DVE, concurrent with ACT's ktile-a zz
                nc.vector.tensor_tensor(
                    zz[:], t[0:kk, cb : cb + 2 * Wl], t[0:kk, cb : cb + 2 * Wl],
                    AluOp.mult,
                )
            # Pool for L0 (big) and L3 (Pool is idle by then; DVE is running
            # pw_L2 and would stall b3v); DVE for L1/L2 where Pool runs pw0
            z12_eng = nc.gpsimd if l in (0, 3) else nc.vector
            z12_eng.tensor_tensor(
                z12[:], t[0:kk, cb : cb + Wl], t[0:kk, cb + Wl : cb + 2 * Wl],
                AluOp.mult,
            )
            zz_t.append(zz)
            z12_t.append(z12)

        def msrc(mi, kidx):
            kk, cb = ktiles[kidx]
            zz, z12 = zz_t[kidx], z12_t[kidx]
            return [
                t[0:kk, cb : cb + Wl],
                t[0:kk, cb + Wl : cb + 2 * Wl],
                zz[:, 0:Wl],
                zz[:, Wl : 2 * Wl],
                z12[:],
            ][mi]

        ncw = max(1, Wl // 128)
        cwid = min(128, Wl)
        nkp_n = NKP[l + 1] if fuse_ds else 0

        def mwidth(mi):  # x maps also carry the fused dv columns
            return Rl + (nkp_n if mi < 2 else 0)

        vts = [None] * 5
        for g in groups:
            gw = sum(ncw * mwidth(mi) for mi in g)
            vt_ps = ps_work.tile([128, gw], F32, tag="work", name="vt_ps")
            base = 0
            for mi in g:
                w = mwidth(mi)
                for c in range(ncw):
                    for kidx in range(len(ktiles)):
                        nc.tensor.matmul(
                            vt_ps[0:cwid, base + c * w : base + (c + 1) * w],
                            msrc(mi, kidx)[:, c * cwid : c * cwid + cwid],
                            av_ap(
                                kidx,
                                ktiles[kidx][0],
                                AV_OFF[l],
                                AV_OFF[l] + w,
                            ),
                            start=(kidx == 0),
                            stop=(kidx == len(ktiles) - 1),
                        )
                base += ncw * w
            vt_sb = sb2.tile([128, gw], DT_BOX, tag="vt_sb", name="vt_sb")
            copy_cast(vt_sb[0:cwid, :], vt_ps[0:cwid, :])
            base = 0
            for mi in g:
                w = mwidth(mi)
                vts[mi] = (vt_sb[:, base : base + ncw * w], w)
                base += ncw * w
        return vts

    def box_h(l, vts):
        """Horizontal pass: stationary = vT chunk, mobile = box band."""
        Wl, Rl = H[l], R[l]
        cwid = min(128, Wl)
        box_ps = {}
        for mi in range(5):
            if l == 0:
                bp = ps_box.tile([Rl, Wl], F32, tag="box", name="bp")
                out_base = 0
            else:
                bp = box_deep[mi]
                out_base = deep_off[l]
            vt, st = vts[mi]
            for (j, lo, hi), off in zip(BH_CH[l], BH_OFF[l]):
                nc.tensor.matmul(
                    bp[0:Rl, out_base + lo : out_base + hi],
                    vt[0:cwid, j * st : j * st + Rl],
                    bh_ap(cwid, off, off + (hi - lo)),
                    start=(j == 0),
                    stop=(j == len(BH_CH[l]) - 1),
                )
            box_ps[mi] = bp
        return box_ps

    def ds_h(l, vts):
        """Downsample horizontal pass off the fused box_v output (the dv
        columns ride along in the x maps' vT chunks at offset Rl)."""
        Wl, Rl, nkn, nkp = H[l], R[l], NK[l + 1], NKP[l + 1]
        rch = Wl // 128
        xnext_ps = ps_work.tile([nkp, 2 * (Wl // 2)], F32, tag="work", name="xn_ps")
        for j in range(2 * Wl // 128):
            img, jr = j // rch, j % rch
            (jj, lo, hi) = DH_CH[l][jr]
            off = DH_OFF[l][jr]
            vt, st = vts[img]
            nc.tensor.matmul(
                xnext_ps[:, img * (Wl // 2) + lo : img * (Wl // 2) + hi],
                vt[0:128, jr * st + Rl : jr * st + Rl + nkp],
                dh_ap(128, off, off + (hi - lo)),
                start=(jr == 0),
                stop=(jr == rch - 1),
            )
        copy_cast(xt[l + 1][:], xnext_ps[0:nkn, :])

    def pointwise(box, Rl, Wl, lvls, clamp=False, pp_eng=None, m2c_eng=None):
        """box: dict mi-> PSUM AP rect [Rl, Wl]; lvls: list of
        (level, part_rows, col_lo, col_hi) for the ttr accumulations.
        zz maps are unscaled x*x, so sig1/sig2 fold the x121 here."""
        pp_eng = pp_eng or nc.gpsimd
        m1, m2, r11, r22, r12 = (box[i] for i in range(5))
        q1 = sb2.tile([Rl, Wl], F32, tag="q1")
        q2 = sb2.tile([Rl, Wl], F32, tag="q2")
        sig1 = sb2.tile([Rl, Wl], F32, tag="sig1")
        sig2 = sb2.tile([Rl, Wl], F32, tag="sig2")
        q12 = sb2.tile([Rl, Wl], F32, tag="q12")
        sig12 = sb2.tile([Rl, Wl], F32, tag="sig12")
        pp = sb2.tile([Rl, Wl], F32, tag="pp")
        rr = sb2.tile([Rl, Wl], F32, tag="rr")
        cs = sb2.tile([Rl, Wl], F32, tag="cs")

        m2c = sb2.tile([Rl, Wl], F32, tag="m2c")
        nc.scalar.activation(q1[:], m1, ActFn.Square)
        nc.scalar.activation(q2[:], m2, ActFn.Square)
        # only one operand of a DVE op may live in PSUM -> stage m2 in SBUF
        nc.scalar.activation(m2c[:], m2, ActFn.Copy)
        nc.vector.scalar_tensor_tensor(
            sig1[:], r11, 121.0, q1[:], AluOp.mult, AluOp.subtract
        )
        nc.vector.scalar_tensor_tensor(
            sig2[:], r22, 121.0, q2[:], AluOp.mult, AluOp.subtract
        )
        nc.vector.tensor_tensor(q12[:], m1, m2c[:], AluOp.mult)
        nc.vector.scalar_tensor_tensor(
            sig12[:], r12, 121.0, q12[:], AluOp.mult, AluOp.subtract
        )
        pp_eng.tensor_tensor(pp[:], sig1[:], sig2[:], AluOp.mult)
        if clamp:
            # keep unused (never-reduced) lanes finite through rsqrt
            nc.vector.tensor_scalar_max(pp[:], pp[:], 1e-20)
        # 1/sqrt(|pp|) in one ACT op (same table set as Square/Copy)
        nc.scalar.activation(rr[:], pp[:], ActFn.Abs_reciprocal_sqrt)
        for (lv, pr, clo, chi) in lvls:
            # C = sig12*r summed along the free axis; tensor_tensor_reduce
            # crashes the device (NRT unrecoverable), stt+accum_out works
            nc.vector.scalar_tensor_tensor(
                cs[0:pr, clo:chi],
                sig12[0:pr, clo:chi],
                1.0,
                rr[0:pr, clo:chi],
                AluOp.mult,
                AluOp.mult,
                accum_out=acc[0:pr, lv : lv + 1],
            )

    def downsample(l):
        """xt[l] -> xt[l+1]: transposed-output vertical stride-2, then dh."""
        Wl, nk, nkn, nkp = H[l], NK[l], NK[l + 1], NKP[l + 1]
        t = xt[l]
        ktiles = KT[l]
        # vertical: stationary = x chunk, mobile = Dv [K, nkp]; vT chunks land
        # in PSUM in pieces to bound bank usage
        nch = 2 * Wl // 128
        npieces = 2 if Wl >= 512 else 1
        chpp = nch // npieces
        vt_sb = sb2.tile([128, nch * nkp], DT_BOX, tag="vt_sb", name="vt_sb")
        for pc in range(npieces):
            vt_ps = ps_work.tile([128, chpp * nkp], F32, tag="work", name="vt_ps")
            for cc in range(chpp):
                c = pc * chpp + cc
                for kidx in range(len(ktiles)):
                    kk, cb = ktiles[kidx]
                    nc.tensor.matmul(
                        vt_ps[:, cc * nkp : (cc + 1) * nkp],
                        t[0:kk, cb + c * 128 : cb + (c + 1) * 128],
                        dv_ap(kidx, kk, DV_OFF[l], DV_OFF[l] + nkp),
                        start=(kidx == 0),
                        stop=(kidx == len(ktiles) - 1),
                    )
            copy_cast(
                vt_sb[:, pc * chpp * nkp : (pc + 1) * chpp * nkp], vt_ps[:]
            )
        # horizontal: stationary = vT chunk [128, nkp], mobile = Dh window
        rch = Wl // 128
        xnext_ps = ps_work.tile([nkp, 2 * (Wl // 2)], F32, tag="work", name="xn_ps")
        for j in range(2 * Wl // 128):
            reg, jr = j // rch, j % rch
            (jj, lo, hi) = DH_CH[l][jr]
            off = DH_OFF[l][jr]
            nc.tensor.matmul(
                xnext_ps[:, reg * (Wl // 2) + lo : reg * (Wl // 2) + hi],
                vt_sb[:, j * nkp : (j + 1) * nkp],
                dh_ap(128, off, off + (hi - lo)),
                start=(jr == 0),
                stop=(jr == rch - 1),
            )
        copy_cast(xt[l + 1][:], xnext_ps[0:nkn, :])
        return xnext_ps

    # ---------------- main schedule ----------------
    # downsample chain first (it is the critical path into the deep levels);
    # level-0 box + pointwise fill the other engines behind it
    xn0 = downsample(0)
    b0v = box_v(0, ((0,), (1,), (2,), (3,), (4,)))
    box0 = box_h(0, b0v)
    # all deep V-passes are emitted before any pointwise: their DVE/ACT/Pool
    # dependencies must not queue behind the long pointwise chains
    b1v = box_v(1, ((0, 1), (2, 3), (4,)), fuse_ds=True, xn_ps=xn0)
    xn1 = ds_h(1, b1v)
    b2v = box_v(2, ((0, 1, 2, 3, 4),), fuse_ds=True, xn_ps=xn1)
    xn2 = ds_h(2, b2v)
    b3v = box_v(3, ((0, 1, 2, 3, 4),), xn_ps=xn2)
    pointwise(
        {i: box0[i][:, :] for i in range(5)},
        128,
        512,
        [(0, 128, 0, 512)],
        pp_eng=nc.gpsimd,
        m2c_eng=nc.gpsimd,
    )

    box_deep = [
        ps_box.tile([128, 448], F32, tag="box", name=f"boxdeep{m}") for m in range(5)
    ]
    box_h(1, b1v)
    pointwise(
        {i: box_deep[i][0:64, 0:256] for i in range(5)},
        64,
        256,
        [(1, 64, 0, 256)],
        pp_eng=nc.vector,
        m2c_eng=nc.scalar,
    )
    box_h(2, b2v)
    pointwise(
        {i: box_deep[i][0:32, 256:384] for i in range(5)},
        32,
        128,
        [(2, 32, 0, 128)],
        pp_eng=nc.vector,
        m2c_eng=nc.scalar,
    )
    box_h(3, b3v)
    pointwise(
        {i: box_deep[i][0:16, 384:448] for i in range(5)},
        16,
        64,
        [(3, 16, 0, 64)],
        pp_eng=nc.vector,
        m2c_eng=nc.scalar,
    )

    nc.sync.dma_start(outp[:], acc[:])


# ----------------------------------------------------------------------------
# public entry point
# ----------------------------------------------------------------------------
_NC_CACHE = {}


def _get_program():
    if "nc" not in _NC_CACHE:
        _NC_CACHE["nc"] = build_program()
    return _NC_CACHE["nc"]


def _core_inputs(img1, img2):
    if "shared" not in _NC_CACHE:
        bhm, dhm = _build_shared_mats()
        _NC_CACHE["shared"] = np.ascontiguousarray(
            np.concatenate([bhm, dhm], axis=1)
        )
        _NC_CACHE["core"] = [
            np.ascontiguousarray(_pack_band_rows(_build_core_mats(i)))
            for i in range(4)
        ]
    maps = []
    for c in range(8):
        b, i = c // 4, c % 4
        maps.append(
            {
                "ximg": _band_slices(img1, img2, b, i),
                "avdm": _NC_CACHE["core"][i],
                "bhdm": _NC_CACHE["shared"],
            }
        )
    return maps


def _finish(results):
    total = 0.0
    for l in range(NLVL):
        s = 0.0
        for c in range(8):
            s += float(np.sum(results[c]["out"][0 : R[l], l].astype(np.float64)))
        mean_c = s / (2.0 * H[l] * H[l])
        total += PYR_W[l] * (2.0 - 2.0 * mean_c)
    return np.float32(total)


def kernel(img1, img2, _run_kwargs=None):
    img1 = np.asarray(img1, np.float32)
    img2 = np.asarray(img2, np.float32)
    nc = _get_program()
    in_maps = _core_inputs(img1, img2)
    res = run_bass_kernel_spmd(nc, in_maps, list(range(8)), **(_run_kwargs or {}))
    out = _finish(res.results)
    if _run_kwargs:
        return out, res
    return out



# revision 71
# speedup vs baseline: 1.0314x; 1.0314x over previous
"""SSIM-pyramid loss kernel for 8 Trainium2 NeuronCores (Bass/Tile).

Math: the reference loss per pyramid level reduces EXACTLY (to ~1e-8 rel) to
    loss_l = 2 - 2*mean(sig12 / (sqrt(sig1+eps)*sqrt(sig2+eps)))
because sum_k n1^2 over a window = 121*sig1/s1^2 ~= 121*(1 - O(eps/sig)),
with eps=1e-10 and sig >= 3e-3 for these inputs.  So per level we need only
5 box-filtered maps: box(x1), box(x2), box(x1^2), box(x2^2), box(x1*x2).

Distribution: batch b = core//4, row-band i = core%4 (128 rows of L0 per
core).  Each core computes its band of all 4 pyramid levels from a padded
222-row slice of the level-0 images, using per-core banded matrices (inputs)
that encode box-filter truncation and bicubic edge clamping.  Per-core
partial sums are combined on the host.

Box filters / downsamples run on the TensorEngine as banded matmuls; every
vertical pass uses stationary=data-chunk matmuls that emit the transposed
intermediate directly in PSUM (no DMA/xbar transposes anywhere); at levels
1/2 the fused av|dv mobile makes one V pass feed both box and downsample.
Pointwise math uses fused DVE ops (scalar_tensor_tensor with accum_out) and
a single Abs_reciprocal_sqrt activation; its table set is pre-warmed during
the input DMAs.  Inputs are packed into 128-row tiles so every DMA spreads
across all 16 DMA engines.
"""

import sys

sys.path.insert(0, "/opt/trn_rl_repo")

import numpy as np
import ml_dtypes

import concourse.bass as bass  # noqa: E402
import concourse.mybir as mybir  # noqa: E402
import concourse.tile as tile  # noqa: E402
from concourse import bacc  # noqa: E402
from concourse.bass_utils import run_bass_kernel_spmd  # noqa: E402

F32 = mybir.dt.float32
BF16 = mybir.dt.bfloat16
FP16 = mybir.dt.float16

# Whole PE path runs fp16: fp32 matmuls cost 2x (HI/LO passes), bf16 loses
# ~2e-3 accuracy through the sig cancellations, fp16 loses only ~1.6e-4 and
# enables the 2-byte DMA xbar transpose.  PSUM accumulation and the pointwise
# chain stay fp32.
DT_BOX = FP16
NP_BOX = np.float16

WS, PAD = 11, 5
BIC = np.array([-0.09375, 0.59375, 0.59375, -0.09375], np.float64)
PYR_W = (0.2, 0.4, 0.6, 0.8)
NLVL = 4
H = [512, 256, 128, 64]  # = W per level
R = [128, 64, 32, 16]  # band rows per core per level
AluOp = mybir.AluOpType
ActFn = mybir.ActivationFunctionType


# ----------------------------------------------------------------------------
# geometry
# ----------------------------------------------------------------------------
def _lr_ranges(i):
    """Row ranges (unclamped, fixed size) each core carries per level."""
    lr = [None] * NLVL
    s3 = 16 * i
    lr[3] = (s3 - PAD, s3 + 16 + PAD)
    for l in (2, 1, 0):
        s = R[l] * i
        box = (s - PAD, s + R[l] + PAD)
        a1, b1 = lr[l + 1]
        ds = (2 * a1 - 1, 2 * (b1 - 1) + 2 + 1)  # taps 2j-1..2j+2 for j in lr[l+1]
        lr[l] = (min(box[0], ds[0]), max(box[1], ds[1]))
    return lr


NK = [222, 110, 54, 26]  # sizes of lr ranges (identical for all cores)
for _i in range(4):
    _lr = _lr_ranges(_i)
    assert [b - a for a, b in _lr] == NK, (_i, _lr)

# horizontal box-filter chunking: aligned 128 chunks, chunk0 streams full W
# (chunk0's full-width write also resets the PSUM accumulation group); each
# level's pointwise reads only the lanes its own box_h wrote, so no extra
# init streaming is needed
def _bh_windows(W_, full0=None):
    ch = []
    ncw = max(1, W_ // 128)
    for j in range(ncw):
        if j == 0:
            ch.append((0, 0, full0 or W_))
        else:
            ch.append((j, 128 * j - PAD, min(W_, 128 * j + 128 + PAD)))
    return ch


def _dh_windows(W_):  # per-region in-chunks for stride-2 4-tap downsample
    Wn = W_ // 2
    ch = []
    ncw = max(1, W_ // 128)
    for j in range(ncw):
        if j == 0:
            ch.append((0, 0, Wn))
        else:
            ch.append((j, 64 * j - 1, min(Wn, 64 * j + 65)))
    return ch


BH_CH = [_bh_windows(H[l]) for l in range(NLVL)]
DH_CH = [_dh_windows(H[l]) for l in range(NLVL - 1)]
BH_OFF, _o = [], 0
for l in range(NLVL):
    offs = []
    for (_, lo, hi) in BH_CH[l]:
        offs.append(_o)
        _o += hi - lo
    BH_OFF.append(offs)
BH_COLS = _o
DH_OFF, _o = [], 0
for l in range(NLVL - 1):
    offs = []
    for (_, lo, hi) in DH_CH[l]:
        offs.append(_o)
        _o += hi - lo
    DH_OFF.append(offs)
DH_COLS = _o

NKP = [None, 112, 64, 32]  # Dv output rows padded to a multiple of 16
# fused constant layout: [av0 | av1 dv1 | av2 dv2 | av3 | dv0]; av_l and dv_l
# are adjacent for l=1,2 so one V matmul serves box and downsample
AV_OFF = [0, 128, 256, 320]
DV_OFF = [336, 192, 288]
AVD_COLS = 448


# ----------------------------------------------------------------------------
# host-side per-core constant matrices
# ----------------------------------------------------------------------------
def _build_core_mats(i):
    lr = _lr_ranges(i)

    avd = np.zeros((NK[0], AVD_COLS), np.float64)
    for l in range(NLVL):
        a, _b = lr[l]
        s = R[l] * i
        for k in range(NK[l]):
            g = a + k
            for m in range(R[l]):
                if abs(g - (s + m)) <= PAD:
                    avd[k, AV_OFF[l] + m] = 1.0

    for l in range(NLVL - 1):
        a, _b = lr[l]
        an, bn = lr[l + 1]
        for m in range(NK[l + 1]):
            j = an + m
            if j < 0 or j >= H[l + 1]:
                continue
            for t in range(4):
                src = min(max(2 * j - 1 + t, 0), H[l] - 1)
                k = src - a
                assert 0 <= k < NK[l], (l, i, j, src)
                avd[k, DV_OFF[l] + m] += BIC[t]

    return avd.astype(NP_BOX)  # taps 1.0 / BIC exact in fp16


def _build_shared_mats():
    bhm = np.zeros((128, BH_COLS), np.float64)
    for l in range(NLVL):
        for (j, lo, hi), off in zip(BH_CH[l], BH_OFF[l]):
            base = 128 * j
            for p in range(min(128, H[l] - base)):
                w = base + p
                for wp in range(lo, hi):
                    if abs(w - wp) <= PAD:
                        bhm[p, off + (wp - lo)] = 1.0

    dhm = np.zeros((128, DH_COLS), np.float64)
    for l in range(NLVL - 1):
        for (j, lo, hi), off in zip(DH_CH[l], DH_OFF[l]):
            base = 128 * j
            for wp in range(lo, hi):
                for t in range(4):
                    src = min(max(2 * wp - 1 + t, 0), H[l] - 1)
                    p = src - base
                    if 0 <= p < 128:
                        dhm[p, off + (wp - lo)] += BIC[t]
    return bhm.astype(NP_BOX), dhm.astype(NP_BOX)


def _band_slices(img1, img2, b, i):
    """[128, 2048] packed band: rows 0:128 in cols 0:1024, rows 128:222 in
    cols 1024:2048 (so the DMA spreads across all 16 engines)."""
    a, e = _lr_ranges(i)[0]
    band = np.zeros((NK[0], 1024), np.float32)
    lo, hi = max(a, 0), min(e, 512)
    band[lo - a : hi - a, 0:512] = img1[b, 0, lo:hi, :]
    band[lo - a : hi - a, 512:1024] = img2[b, 0, lo:hi, :]
    out = np.zeros((128, 2048), np.float32)
    out[0:128, 0:1024] = band[0:128]
    out[0 : NK[0] - 128, 1024:2048] = band[128:]
    return out.astype(NP_BOX)


def _pack_band_rows(m):
    """[222, C] -> [128, 2C]: rows 128:222 packed into the right half."""
    c = m.shape[1]
    out = np.zeros((128, 2 * c), m.dtype)
    out[0:128, 0:c] = m[0:128]
    out[0 : m.shape[0] - 128, c:] = m[128:]
    return out


# ----------------------------------------------------------------------------
# device program
# ----------------------------------------------------------------------------
BHD_COLS = BH_COLS + DH_COLS  # bh | dh fused side by side


def build_program():
    nc = bacc.Bacc("TRN2", target_bir_lowering=False)

    # band rows 128:222 are packed into the right half of the 128-partition
    # tiles: DMAs with >=128 rows of >=1.7KB spread over all 16 DMA engines,
    # while a 94-row DMA lands on a single engine (~10x slower)
    ximg = nc.dram_tensor("ximg", [128, 2048], DT_BOX, kind="ExternalInput")
    avdm = nc.dram_tensor("avdm", [128, 2 * AVD_COLS], DT_BOX, kind="ExternalInput")
    bhdm = nc.dram_tensor("bhdm", [128, BHD_COLS], DT_BOX, kind="ExternalInput")
    outp = nc.dram_tensor("out", [128, 4], F32, kind="ExternalOutput")

    with tile.TileContext(nc) as tc:
        with (
            tc.tile_pool(name="sb1", bufs=1) as sb1,
            tc.tile_pool(name="sb2", bufs=2) as sb2,
            tc.tile_pool(name="ps_box", bufs=5, space="PSUM") as ps_box,
            tc.tile_pool(name="ps_work", bufs=3, space="PSUM") as ps_work,
        ):
            _emit(nc, tc, sb1, sb2, ps_box, ps_work, ximg, avdm, bhdm, outp)

    nc.compile()
    return nc


def _emit(nc, tc, sb1, sb2, ps_box, ps_work, ximg, avdm, bhdm, outp):
    # ---- load constants & input band -------------------------------------
    avd = sb1.tile([128, 2 * AVD_COLS], DT_BOX, tag="avd")
    bhd = sb1.tile([128, BHD_COLS], DT_BOX, tag="bhd")
    xt0 = sb1.tile([128, 2048], DT_BOX, tag="xt0")
    # >=128-row DMAs; each spreads over all 16 DMA engines.  Left band half
    # and avd first (they gate the first V matmuls); bhd (needed last, by the
    # H passes) queues behind both ximg halves
    # transfers within one queue complete in order; avd + right half finish
    # before the left half does, so the V matmuls (which need all three) run
    # with no ktile-b stall once the left half lands
    nc.sync.dma_start(avd[:], avdm[:])
    nc.scalar.dma_start(xt0[:, 1024:2048], ximg[:, 1024:2048])
    nc.sync.dma_start(xt0[:, 0:1024], ximg[:, 0:1024])
    nc.scalar.dma_start(bhd[:], bhdm[:])

    def av_ap(kidx, kk, c0, c1):
        return avd[0:kk, kidx * AVD_COLS + c0 : kidx * AVD_COLS + c1]

    def dv_ap(kidx, kk, c0, c1):
        return avd[0:kk, kidx * AVD_COLS + c0 : kidx * AVD_COLS + c1]

    def bh_ap(p, c0, c1):
        return bhd[0:p, c0:c1]

    def dh_ap(p, c0, c1):
        return bhd[0:p, BH_COLS + c0 : BH_COLS + c1]

    acc = sb1.tile([128, 4], F32, tag="acc")
    nc.vector.memset(acc[:], 0.0)

    # warm the abs_reciprocal_sqrt ACT table set (it also contains Square and
    # Copy) while input DMAs stream, so no 1.3us table reload fires mid-kernel
    warm = sb1.tile([128, 1], F32, tag="warm")
    nc.vector.memset(warm[:], 1.0)
    nc.scalar.activation(warm[:], warm[:], ActFn.Abs_reciprocal_sqrt)

    # per-level x tiles (levels 1..3 produced on-chip); level 0 is packed
    # [128, 2048] with band rows 128:222 in columns 1024:2048
    xt = [
        xt0,
        sb1.tile([NK[1], 512], DT_BOX, tag="xt1", name="xt1"),
        sb1.tile([NK[2], 256], DT_BOX, tag="xt2", name="xt2"),
        sb1.tile([NK[3], 128], DT_BOX, tag="xt3", name="xt3"),
    ]
    # (rows, col_base) sub-bands of each level's tile
    KT = [
        [(128, 0), (NK[0] - 128, 1024)],
        [(NK[1], 0)],
        [(NK[2], 0)],
        [(NK[3], 0)],
    ]

    # deep-level box maps parked in PSUM: map -> [128, 448] tile
    # L1 at [0:64,0:256], L2 at [0:32,256:384], L3 at [0:16,384:448]
    deep_off = {1: 0, 2: 256, 3: 384}
    deep_w = {1: 256, 2: 128, 3: 64}
    box_deep = None

    copy_rr = [0]

    def copy_cast(dst_ap, src_ap):
        # PSUM->SBUF copies alternate between DVE and ACT
        if copy_rr[0] % 2 == 0:
            nc.vector.tensor_copy(dst_ap, src_ap)
        else:
            nc.scalar.activation(dst_ap, src_ap, ActFn.Copy)
        copy_rr[0] += 1

    def box_v(l, groups, fuse_ds=False, xn_ps=None):
        """z-maps + transposed-output vertical pass for level l.  groups:
        tuples of map indices sharing one PSUM tile and one copy_cast.
        With fuse_ds the x maps' mobile is the fused av|dv block, producing
        box-V and downsample-V in one matmul.  Returns per-map (vT AP,
        chunk stride)."""
        Wl, Rl, nk = H[l], R[l], NK[l]
        t = xt[l]
        ktiles = KT[l]

        # z-maps: zz = x*x unscaled (the x121 is folded into the pointwise
        # stt ops).  Deep levels square the downsample PSUM directly on ACT,
        # overlapping the xt copy instead of waiting for it.
        zz_t, z12_t = [], []
        for (kk, cb) in ktiles:
            zz = sb2.tile([kk, 2 * Wl], DT_BOX, tag=f"zz{len(zz_t)}", name="zz")
            z12 = sb2.tile([kk, Wl], DT_BOX, tag=f"z12{len(z12_t)}", name="z12")
            if xn_ps is not None:
                nc.scalar.activation(
                    zz[:], xn_ps[0:kk, 0 : 2 * Wl], ActFn.Square
                )
            else:
                # ACT, not DVE: the Vector queue is the busiest engine and
                # ACT is idle while the input band streams in
                nc.scalar.activation(
                    zz[:], t[0:kk, cb : cb + 2 * Wl], ActFn.Square
                )
            # Pool for L0 (big) and L3 (Pool is idle by then; DVE is running
            # pw_L2 and would stall b3v); DVE for L1/L2 where Pool runs pw0
            z12_eng = nc.gpsimd if l in (0, 3) else nc.vector
            z12_eng.tensor_tensor(
                z12[:], t[0:kk, cb : cb + Wl], t[0:kk, cb + Wl : cb + 2 * Wl],
                AluOp.mult,
            )
            zz_t.append(zz)
            z12_t.append(z12)

        def msrc(mi, kidx):
            kk, cb = ktiles[kidx]
            zz, z12 = zz_t[kidx], z12_t[kidx]
            return [
                t[0:kk, cb : cb + Wl],
                t[0:kk, cb + Wl : cb + 2 * Wl],
                zz[:, 0:Wl],
                zz[:, Wl : 2 * Wl],
                z12[:],
            ][mi]

        ncw = max(1, Wl // 128)
        cwid = min(128, Wl)
        nkp_n = NKP[l + 1] if fuse_ds else 0

        def mwidth(mi):  # x maps also carry the fused dv columns
            return Rl + (nkp_n if mi < 2 else 0)

        vts = [None] * 5
        for g in groups:
            gw = sum(ncw * mwidth(mi) for mi in g)
            vt_ps = ps_work.tile([128, gw], F32, tag="work", name="vt_ps")
            base = 0
            for mi in g:
                w = mwidth(mi)
                for c in range(ncw):
                    for kidx in range(len(ktiles)):
                        nc.tensor.matmul(
                            vt_ps[0:cwid, base + c * w : base + (c + 1) * w],
                            msrc(mi, kidx)[:, c * cwid : c * cwid + cwid],
                            av_ap(
                                kidx,
                                ktiles[kidx][0],
                                AV_OFF[l],
                                AV_OFF[l] + w,
                            ),
                            start=(kidx == 0),
                            stop=(kidx == len(ktiles) - 1),
                        )
                base += ncw * w
            vt_sb = sb2.tile([128, gw], DT_BOX, tag="vt_sb", name="vt_sb")
            copy_cast(vt_sb[0:cwid, :], vt_ps[0:cwid, :])
            base = 0
            for mi in g:
                w = mwidth(mi)
                vts[mi] = (vt_sb[:, base : base + ncw * w], w)
                base += ncw * w
        return vts

    def box_h(l, vts):
        """Horizontal pass: stationary = vT chunk, mobile = box band."""
        Wl, Rl = H[l], R[l]
        cwid = min(128, Wl)
        box_ps = {}
        for mi in range(5):
            if l == 0:
                bp = ps_box.tile([Rl, Wl], F32, tag="box", name="bp")
                out_base = 0
            else:
                bp = box_deep[mi]
                out_base = deep_off[l]
            vt, st = vts[mi]
            for (j, lo, hi), off in zip(BH_CH[l], BH_OFF[l]):
                nc.tensor.matmul(
                    bp[0:Rl, out_base + lo : out_base + hi],
                    vt[0:cwid, j * st : j * st + Rl],
                    bh_ap(cwid, off, off + (hi - lo)),
                    start=(j == 0),
                    stop=(j == len(BH_CH[l]) - 1),
                )
            box_ps[mi] = bp
        return box_ps

    def ds_h(l, vts):
        """Downsample horizontal pass off the fused box_v output (the dv
        columns ride along in the x maps' vT chunks at offset Rl)."""
        Wl, Rl, nkn, nkp = H[l], R[l], NK[l + 1], NKP[l + 1]
        rch = Wl // 128
        xnext_ps = ps_work.tile([nkp, 2 * (Wl // 2)], F32, tag="work", name="xn_ps")
        for j in range(2 * Wl // 128):
            img, jr = j // rch, j % rch
            (jj, lo, hi) = DH_CH[l][jr]
            off = DH_OFF[l][jr]
            vt, st = vts[img]
            nc.tensor.matmul(
                xnext_ps[:, img * (Wl // 2) + lo : img * (Wl // 2) + hi],
                vt[0:128, jr * st + Rl : jr * st + Rl + nkp],
                dh_ap(128, off, off + (hi - lo)),
                start=(jr == 0),
                stop=(jr == rch - 1),
            )
        copy_cast(xt[l + 1][:], xnext_ps[0:nkn, :])

    def pointwise(box, Rl, Wl, lvls, clamp=False, pp_eng=None, m2c_eng=None):
        """box: dict mi-> PSUM AP rect [Rl, Wl]; lvls: list of
        (level, part_rows, col_lo, col_hi) for the ttr accumulations.
        zz maps are unscaled x*x, so sig1/sig2 fold the x121 here."""
        pp_eng = pp_eng or nc.gpsimd
        m1, m2, r11, r22, r12 = (box[i] for i in range(5))
        q1 = sb2.tile([Rl, Wl], F32, tag="q1")
        q2 = sb2.tile([Rl, Wl], F32, tag="q2")
        sig1 = sb2.tile([Rl, Wl], F32, tag="sig1")
        sig2 = sb2.tile([Rl, Wl], F32, tag="sig2")
        q12 = sb2.tile([Rl, Wl], F32, tag="q12")
        sig12 = sb2.tile([Rl, Wl], F32, tag="sig12")
        pp = sb2.tile([Rl, Wl], F32, tag="pp")
        rr = sb2.tile([Rl, Wl], F32, tag="rr")
        cs = sb2.tile([Rl, Wl], F32, tag="cs")

        m2c = sb2.tile([Rl, Wl], F32, tag="m2c")
        nc.scalar.activation(q1[:], m1, ActFn.Square)
        nc.scalar.activation(q2[:], m2, ActFn.Square)
        # only one operand of a DVE op may live in PSUM -> stage m2 in SBUF
        nc.scalar.activation(m2c[:], m2, ActFn.Copy)
        nc.vector.scalar_tensor_tensor(
            sig1[:], r11, 121.0, q1[:], AluOp.mult, AluOp.subtract
        )
        nc.vector.scalar_tensor_tensor(
            sig2[:], r22, 121.0, q2[:], AluOp.mult, AluOp.subtract
        )
        nc.vector.tensor_tensor(q12[:], m1, m2c[:], AluOp.mult)
        nc.vector.scalar_tensor_tensor(
            sig12[:], r12, 121.0, q12[:], AluOp.mult, AluOp.subtract
        )
        pp_eng.tensor_tensor(pp[:], sig1[:], sig2[:], AluOp.mult)
        if clamp:
            # keep unused (never-reduced) lanes finite through rsqrt
            nc.vector.tensor_scalar_max(pp[:], pp[:], 1e-20)
        # 1/sqrt(|pp|) in one ACT op (same table set as Square/Copy)
        nc.scalar.activation(rr[:], pp[:], ActFn.Abs_reciprocal_sqrt)
        for (lv, pr, clo, chi) in lvls:
            # C = sig12*r summed along the free axis; tensor_tensor_reduce
            # crashes the device (NRT unrecoverable), stt+accum_out works
            nc.vector.scalar_tensor_tensor(
                cs[0:pr, clo:chi],
                sig12[0:pr, clo:chi],
                1.0,
                rr[0:pr, clo:chi],
                AluOp.mult,
                AluOp.mult,
                accum_out=acc[0:pr, lv : lv + 1],
            )

    def downsample(l):
        """xt[l] -> xt[l+1]: transposed-output vertical stride-2, then dh."""
        Wl, nk, nkn, nkp = H[l], NK[l], NK[l + 1], NKP[l + 1]
        t = xt[l]
        ktiles = KT[l]
        # vertical: stationary = x chunk, mobile = Dv [K, nkp]; vT chunks land
        # in PSUM in pieces to bound bank usage
        nch = 2 * Wl // 128
        npieces = 2 if Wl >= 512 else 1
        chpp = nch // npieces
        vt_sb = sb2.tile([128, nch * nkp], DT_BOX, tag="vt_sb", name="vt_sb")
        for pc in range(npieces):
            vt_ps = ps_work.tile([128, chpp * nkp], F32, tag="work", name="vt_ps")
            for cc in range(chpp):
                c = pc * chpp + cc
                for kidx in range(len(ktiles)):
                    kk, cb = ktiles[kidx]
                    nc.tensor.matmul(
                        vt_ps[:, cc * nkp : (cc + 1) * nkp],
                        t[0:kk, cb + c * 128 : cb + (c + 1) * 128],
                        dv_ap(kidx, kk, DV_OFF[l], DV_OFF[l] + nkp),
                        start=(kidx == 0),
                        stop=(kidx == len(ktiles) - 1),
                    )
            copy_cast(
                vt_sb[:, pc * chpp * nkp : (pc + 1) * chpp * nkp], vt_ps[:]
            )
        # horizontal: stationary = vT chunk [128, nkp], mobile = Dh window
        rch = Wl // 128
        xnext_ps = ps_work.tile([nkp, 2 * (Wl // 2)], F32, tag="work", name="xn_ps")
        for j in range(2 * Wl // 128):
            reg, jr = j // rch, j % rch
            (jj, lo, hi) = DH_CH[l][jr]
            off = DH_OFF[l][jr]
            nc.tensor.matmul(
                xnext_ps[:, reg * (Wl // 2) + lo : reg * (Wl // 2) + hi],
                vt_sb[:, j * nkp : (j + 1) * nkp],
                dh_ap(128, off, off + (hi - lo)),
                start=(jr == 0),
                stop=(jr == rch - 1),
            )
        copy_cast(xt[l + 1][:], xnext_ps[0:nkn, :])
        return xnext_ps

    # ---------------- main schedule ----------------
    # downsample chain first (it is the critical path into the deep levels);
    # level-0 box + pointwise fill the other engines behind it
    xn0 = downsample(0)
    b0v = box_v(0, ((0,), (1,), (2,), (3,), (4,)))
    box0 = box_h(0, b0v)
    # all deep V-passes are emitted before any pointwise: their DVE/ACT/Pool
    # dependencies must not queue behind the long pointwise chains
    b1v = box_v(1, ((0, 1), (2, 3), (4,)), fuse_ds=True, xn_ps=xn0)
    xn1 = ds_h(1, b1v)
    b2v = box_v(2, ((0, 1, 2, 3, 4),), fuse_ds=True, xn_ps=xn1)
    xn2 = ds_h(2, b2v)
    b3v = box_v(3, ((0, 1, 2, 3, 4),), xn_ps=xn2)
    pointwise(
        {i: box0[i][:, :] for i in range(5)},
        128,
        512,
        [(0, 128, 0, 512)],
        pp_eng=nc.gpsimd,
        m2c_eng=nc.gpsimd,
    )

    box_deep = [
        ps_box.tile([128, 448], F32, tag="box", name=f"boxdeep{m}") for m in range(5)
    ]
    box_h(1, b1v)
    pointwise(
        {i: box_deep[i][0:64, 0:256] for i in range(5)},
        64,
        256,
        [(1, 64, 0, 256)],
        pp_eng=nc.vector,
        m2c_eng=nc.scalar,
    )
    box_h(2, b2v)
    pointwise(
        {i: box_deep[i][0:32, 256:384] for i in range(5)},
        32,
        128,
        [(2, 32, 0, 128)],
        pp_eng=nc.vector,
        m2c_eng=nc.scalar,
    )
    box_h(3, b3v)
    pointwise(
        {i: box_deep[i][0:16, 384:448] for i in range(5)},
        16,
        64,
        [(3, 16, 0, 64)],
        pp_eng=nc.vector,
        m2c_eng=nc.scalar,
    )

    nc.sync.dma_start(outp[:], acc[:])


# ----------------------------------------------------------------------------
# public entry point
# ----------------------------------------------------------------------------
_NC_CACHE = {}


def _get_program():
    if "nc" not in _NC_CACHE:
        _NC_CACHE["nc"] = build_program()
    return _NC_CACHE["nc"]


def _core_inputs(img1, img2):
    if "shared" not in _NC_CACHE:
        bhm, dhm = _build_shared_mats()
        _NC_CACHE["shared"] = np.ascontiguousarray(
            np.concatenate([bhm, dhm], axis=1)
        )
        _NC_CACHE["core"] = [
            np.ascontiguousarray(_pack_band_rows(_build_core_mats(i)))
            for i in range(4)
        ]
    maps = []
    for c in range(8):
        b, i = c // 4, c % 4
        maps.append(
            {
                "ximg": _band_slices(img1, img2, b, i),
                "avdm": _NC_CACHE["core"][i],
                "bhdm": _NC_CACHE["shared"],
            }
        )
    return maps


def _finish(results):
    total = 0.0
    for l in range(NLVL):
        s = 0.0
        for c in range(8):
            s += float(np.sum(results[c]["out"][0 : R[l], l].astype(np.float64)))
        mean_c = s / (2.0 * H[l] * H[l])
        total += PYR_W[l] * (2.0 - 2.0 * mean_c)
    return np.float32(total)


def kernel(img1, img2, _run_kwargs=None):
    img1 = np.asarray(img1, np.float32)
    img2 = np.asarray(img2, np.float32)
    nc = _get_program()
    in_maps = _core_inputs(img1, img2)
    res = run_bass_kernel_spmd(nc, in_maps, list(range(8)), **(_run_kwargs or {}))
    out = _finish(res.results)
    if _run_kwargs:
        return out, res
    return out



# revision 72
# speedup vs baseline: 1.0491x; 1.0172x over previous
"""SSIM-pyramid loss kernel for 8 Trainium2 NeuronCores (Bass/Tile).

Math: the reference loss per pyramid level reduces EXACTLY (to ~1e-8 rel) to
    loss_l = 2 - 2*mean(sig12 / (sqrt(sig1+eps)*sqrt(sig2+eps)))
because sum_k n1^2 over a window = 121*sig1/s1^2 ~= 121*(1 - O(eps/sig)),
with eps=1e-10 and sig >= 3e-3 for these inputs.  So per level we need only
5 box-filtered maps: box(x1), box(x2), box(x1^2), box(x2^2), box(x1*x2).

Distribution: batch b = core//4, row-band i = core%4 (128 rows of L0 per
core).  Each core computes its band of all 4 pyramid levels from a padded
222-row slice of the level-0 images, using per-core banded matrices (inputs)
that encode box-filter truncation and bicubic edge clamping.  Per-core
partial sums are combined on the host.

Box filters / downsamples run on the TensorEngine as banded matmuls; every
vertical pass uses stationary=data-chunk matmuls that emit the transposed
intermediate directly in PSUM (no DMA/xbar transposes anywhere); at levels
1/2 the fused av|dv mobile makes one V pass feed both box and downsample.
Pointwise math uses fused DVE ops (scalar_tensor_tensor with accum_out) and
a single Abs_reciprocal_sqrt activation; its table set is pre-warmed during
the input DMAs.  Inputs are packed into 128-row tiles so every DMA spreads
across all 16 DMA engines.
"""

import sys

sys.path.insert(0, "/opt/trn_rl_repo")

import numpy as np
import ml_dtypes

import concourse.bass as bass  # noqa: E402
import concourse.mybir as mybir  # noqa: E402
import concourse.tile as tile  # noqa: E402
from concourse import bacc  # noqa: E402
from concourse.bass_utils import run_bass_kernel_spmd  # noqa: E402

F32 = mybir.dt.float32
BF16 = mybir.dt.bfloat16
FP16 = mybir.dt.float16

# Whole PE path runs fp16: fp32 matmuls cost 2x (HI/LO passes), bf16 loses
# ~2e-3 accuracy through the sig cancellations, fp16 loses only ~1.6e-4 and
# enables the 2-byte DMA xbar transpose.  PSUM accumulation and the pointwise
# chain stay fp32.
DT_BOX = FP16
NP_BOX = np.float16

WS, PAD = 11, 5
BIC = np.array([-0.09375, 0.59375, 0.59375, -0.09375], np.float64)
PYR_W = (0.2, 0.4, 0.6, 0.8)
NLVL = 4
H = [512, 256, 128, 64]  # = W per level
R = [128, 64, 32, 16]  # band rows per core per level
AluOp = mybir.AluOpType
ActFn = mybir.ActivationFunctionType


# ----------------------------------------------------------------------------
# geometry
# ----------------------------------------------------------------------------
def _lr_ranges(i):
    """Row ranges (unclamped, fixed size) each core carries per level."""
    lr = [None] * NLVL
    s3 = 16 * i
    lr[3] = (s3 - PAD, s3 + 16 + PAD)
    for l in (2, 1, 0):
        s = R[l] * i
        box = (s - PAD, s + R[l] + PAD)
        a1, b1 = lr[l + 1]
        ds = (2 * a1 - 1, 2 * (b1 - 1) + 2 + 1)  # taps 2j-1..2j+2 for j in lr[l+1]
        lr[l] = (min(box[0], ds[0]), max(box[1], ds[1]))
    return lr


NK = [222, 110, 54, 26]  # sizes of lr ranges (identical for all cores)
for _i in range(4):
    _lr = _lr_ranges(_i)
    assert [b - a for a, b in _lr] == NK, (_i, _lr)

# horizontal box-filter chunking: aligned 128 chunks, chunk0 streams full W
# (chunk0's full-width write also resets the PSUM accumulation group); each
# level's pointwise reads only the lanes its own box_h wrote, so no extra
# init streaming is needed
def _bh_windows(W_, full0=None):
    ch = []
    ncw = max(1, W_ // 128)
    for j in range(ncw):
        if j == 0:
            ch.append((0, 0, full0 or W_))
        else:
            ch.append((j, 128 * j - PAD, min(W_, 128 * j + 128 + PAD)))
    return ch


def _dh_windows(W_):  # per-region in-chunks for stride-2 4-tap downsample
    Wn = W_ // 2
    ch = []
    ncw = max(1, W_ // 128)
    for j in range(ncw):
        if j == 0:
            ch.append((0, 0, Wn))
        else:
            ch.append((j, 64 * j - 1, min(Wn, 64 * j + 65)))
    return ch


BH_CH = [_bh_windows(H[l]) for l in range(NLVL)]
DH_CH = [_dh_windows(H[l]) for l in range(NLVL - 1)]
BH_OFF, _o = [], 0
for l in range(NLVL):
    offs = []
    for (_, lo, hi) in BH_CH[l]:
        offs.append(_o)
        _o += hi - lo
    BH_OFF.append(offs)
BH_COLS = _o
DH_OFF, _o = [], 0
for l in range(NLVL - 1):
    offs = []
    for (_, lo, hi) in DH_CH[l]:
        offs.append(_o)
        _o += hi - lo
    DH_OFF.append(offs)
DH_COLS = _o

NKP = [None, 112, 64, 32]  # Dv output rows padded to a multiple of 16
# fused constant layout: [av0 | av1 dv1 | av2 dv2 | av3 | dv0]; av_l and dv_l
# are adjacent for l=1,2 so one V matmul serves box and downsample
AV_OFF = [0, 128, 256, 320]
DV_OFF = [336, 192, 288]
AVD_COLS = 448


# ----------------------------------------------------------------------------
# host-side per-core constant matrices
# ----------------------------------------------------------------------------
def _build_core_mats(i):
    lr = _lr_ranges(i)

    avd = np.zeros((NK[0], AVD_COLS), np.float64)
    for l in range(NLVL):
        a, _b = lr[l]
        s = R[l] * i
        for k in range(NK[l]):
            g = a + k
            for m in range(R[l]):
                if abs(g - (s + m)) <= PAD:
                    avd[k, AV_OFF[l] + m] = 1.0

    for l in range(NLVL - 1):
        a, _b = lr[l]
        an, bn = lr[l + 1]
        for m in range(NK[l + 1]):
            j = an + m
            if j < 0 or j >= H[l + 1]:
                continue
            for t in range(4):
                src = min(max(2 * j - 1 + t, 0), H[l] - 1)
                k = src - a
                assert 0 <= k < NK[l], (l, i, j, src)
                avd[k, DV_OFF[l] + m] += BIC[t]

    return avd.astype(NP_BOX)  # taps 1.0 / BIC exact in fp16


def _build_shared_mats():
    bhm = np.zeros((128, BH_COLS), np.float64)
    for l in range(NLVL):
        for (j, lo, hi), off in zip(BH_CH[l], BH_OFF[l]):
            base = 128 * j
            for p in range(min(128, H[l] - base)):
                w = base + p
                for wp in range(lo, hi):
                    if abs(w - wp) <= PAD:
                        bhm[p, off + (wp - lo)] = 1.0

    dhm = np.zeros((128, DH_COLS), np.float64)
    for l in range(NLVL - 1):
        for (j, lo, hi), off in zip(DH_CH[l], DH_OFF[l]):
            base = 128 * j
            for wp in range(lo, hi):
                for t in range(4):
                    src = min(max(2 * wp - 1 + t, 0), H[l] - 1)
                    p = src - base
                    if 0 <= p < 128:
                        dhm[p, off + (wp - lo)] += BIC[t]
    return bhm.astype(NP_BOX), dhm.astype(NP_BOX)


def _band_slices(img1, img2, b, i):
    """[128, 2048] packed band: rows 0:128 in cols 0:1024, rows 128:222 in
    cols 1024:2048 (so the DMA spreads across all 16 engines)."""
    a, e = _lr_ranges(i)[0]
    band = np.zeros((NK[0], 1024), np.float32)
    lo, hi = max(a, 0), min(e, 512)
    band[lo - a : hi - a, 0:512] = img1[b, 0, lo:hi, :]
    band[lo - a : hi - a, 512:1024] = img2[b, 0, lo:hi, :]
    out = np.zeros((128, 2048), np.float32)
    out[0:128, 0:1024] = band[0:128]
    out[0 : NK[0] - 128, 1024:2048] = band[128:]
    return out.astype(NP_BOX)


def _pack_band_rows(m):
    """[222, C] -> [128, 2C]: rows 128:222 packed into the right half."""
    c = m.shape[1]
    out = np.zeros((128, 2 * c), m.dtype)
    out[0:128, 0:c] = m[0:128]
    out[0 : m.shape[0] - 128, c:] = m[128:]
    return out


# ----------------------------------------------------------------------------
# device program
# ----------------------------------------------------------------------------
BHD_COLS = BH_COLS + DH_COLS  # bh | dh fused side by side


def build_program():
    nc = bacc.Bacc("TRN2", target_bir_lowering=False)

    # band rows 128:222 are packed into the right half of the 128-partition
    # tiles: DMAs with >=128 rows of >=1.7KB spread over all 16 DMA engines,
    # while a 94-row DMA lands on a single engine (~10x slower)
    ximg = nc.dram_tensor("ximg", [128, 2048], DT_BOX, kind="ExternalInput")
    avdm = nc.dram_tensor("avdm", [128, 2 * AVD_COLS], DT_BOX, kind="ExternalInput")
    bhdm = nc.dram_tensor("bhdm", [128, BHD_COLS], DT_BOX, kind="ExternalInput")
    outp = nc.dram_tensor("out", [128, 4], F32, kind="ExternalOutput")

    with tile.TileContext(nc) as tc:
        with (
            tc.tile_pool(name="sb1", bufs=1) as sb1,
            tc.tile_pool(name="sb2", bufs=2) as sb2,
            tc.tile_pool(name="ps_box", bufs=5, space="PSUM") as ps_box,
            tc.tile_pool(name="ps_work", bufs=3, space="PSUM") as ps_work,
        ):
            _emit(nc, tc, sb1, sb2, ps_box, ps_work, ximg, avdm, bhdm, outp)

    nc.compile()
    return nc


def _emit(nc, tc, sb1, sb2, ps_box, ps_work, ximg, avdm, bhdm, outp):
    # ---- load constants & input band -------------------------------------
    avd = sb1.tile([128, 2 * AVD_COLS], DT_BOX, tag="avd")
    bhd = sb1.tile([128, BHD_COLS], DT_BOX, tag="bhd")
    xt0 = sb1.tile([128, 2048], DT_BOX, tag="xt0")
    # >=128-row DMAs; each spreads over all 16 DMA engines.  Left band half
    # and avd first (they gate the first V matmuls); bhd (needed last, by the
    # H passes) queues behind both ximg halves
    # transfers within one queue complete in order; left band half and avd
    # lead their queues (they gate the first V matmuls), bhd trails
    nc.sync.dma_start(xt0[:, 0:1024], ximg[:, 0:1024])
    nc.scalar.dma_start(avd[:], avdm[:])
    nc.scalar.dma_start(xt0[:, 1024:2048], ximg[:, 1024:2048])
    nc.sync.dma_start(bhd[:], bhdm[:])

    def av_ap(kidx, kk, c0, c1):
        return avd[0:kk, kidx * AVD_COLS + c0 : kidx * AVD_COLS + c1]

    def dv_ap(kidx, kk, c0, c1):
        return avd[0:kk, kidx * AVD_COLS + c0 : kidx * AVD_COLS + c1]

    def bh_ap(p, c0, c1):
        return bhd[0:p, c0:c1]

    def dh_ap(p, c0, c1):
        return bhd[0:p, BH_COLS + c0 : BH_COLS + c1]

    acc = sb1.tile([128, 4], F32, tag="acc")
    nc.vector.memset(acc[:], 0.0)

    # warm the abs_reciprocal_sqrt ACT table set (it also contains Square and
    # Copy) while input DMAs stream, so no 1.3us table reload fires mid-kernel
    warm = sb1.tile([128, 1], F32, tag="warm")
    nc.vector.memset(warm[:], 1.0)
    nc.scalar.activation(warm[:], warm[:], ActFn.Abs_reciprocal_sqrt)

    # per-level x tiles (levels 1..3 produced on-chip); level 0 is packed
    # [128, 2048] with band rows 128:222 in columns 1024:2048
    xt = [
        xt0,
        sb1.tile([NK[1], 512], DT_BOX, tag="xt1", name="xt1"),
        sb1.tile([NK[2], 256], DT_BOX, tag="xt2", name="xt2"),
        sb1.tile([NK[3], 128], DT_BOX, tag="xt3", name="xt3"),
    ]
    # (rows, col_base) sub-bands of each level's tile
    KT = [
        [(128, 0), (NK[0] - 128, 1024)],
        [(NK[1], 0)],
        [(NK[2], 0)],
        [(NK[3], 0)],
    ]

    # deep-level box maps parked in PSUM: map -> [128, 448] tile
    # L1 at [0:64,0:256], L2 at [0:32,256:384], L3 at [0:16,384:448]
    deep_off = {1: 0, 2: 256, 3: 384}
    deep_w = {1: 256, 2: 128, 3: 64}
    box_deep = None

    copy_rr = [0]

    def copy_cast(dst_ap, src_ap):
        # PSUM->SBUF copies alternate between DVE and ACT
        if copy_rr[0] % 2 == 0:
            nc.vector.tensor_copy(dst_ap, src_ap)
        else:
            nc.scalar.activation(dst_ap, src_ap, ActFn.Copy)
        copy_rr[0] += 1

    def box_v(l, groups, fuse_ds=False, xn_ps=None):
        """z-maps + transposed-output vertical pass for level l.  groups:
        tuples of map indices sharing one PSUM tile and one copy_cast.
        With fuse_ds the x maps' mobile is the fused av|dv block, producing
        box-V and downsample-V in one matmul.  Returns per-map (vT AP,
        chunk stride)."""
        Wl, Rl, nk = H[l], R[l], NK[l]
        t = xt[l]
        ktiles = KT[l]

        # z-maps: zz = x*x unscaled (the x121 is folded into the pointwise
        # stt ops).  Deep levels square the downsample PSUM directly on ACT,
        # overlapping the xt copy instead of waiting for it.
        zz_t, z12_t = [], []
        for (kk, cb) in ktiles:
            zz = sb2.tile([kk, 2 * Wl], DT_BOX, tag=f"zz{len(zz_t)}", name="zz")
            z12 = sb2.tile([kk, Wl], DT_BOX, tag=f"z12{len(z12_t)}", name="z12")
            if xn_ps is not None:
                nc.scalar.activation(
                    zz[:], xn_ps[0:kk, 0 : 2 * Wl], ActFn.Square
                )
            else:
                # ACT, not DVE: the Vector queue is the busiest engine and
                # ACT is idle while the input band streams in
                nc.scalar.activation(
                    zz[:], t[0:kk, cb : cb + 2 * Wl], ActFn.Square
                )
            # Pool for L0 (big) and L3 (Pool is idle by then; DVE is running
            # pw_L2 and would stall b3v); DVE for L1/L2 where Pool runs pw0
            z12_eng = nc.gpsimd if l in (0, 3) else nc.vector
            z12_eng.tensor_tensor(
                z12[:], t[0:kk, cb : cb + Wl], t[0:kk, cb + Wl : cb + 2 * Wl],
                AluOp.mult,
            )
            zz_t.append(zz)
            z12_t.append(z12)

        def msrc(mi, kidx):
            kk, cb = ktiles[kidx]
            zz, z12 = zz_t[kidx], z12_t[kidx]
            return [
                t[0:kk, cb : cb + Wl],
                t[0:kk, cb + Wl : cb + 2 * Wl],
                zz[:, 0:Wl],
                zz[:, Wl : 2 * Wl],
                z12[:],
            ][mi]

        ncw = max(1, Wl // 128)
        cwid = min(128, Wl)
        nkp_n = NKP[l + 1] if fuse_ds else 0

        def mwidth(mi):  # x maps also carry the fused dv columns
            return Rl + (nkp_n if mi < 2 else 0)

        vts = [None] * 5
        for g in groups:
            gw = sum(ncw * mwidth(mi) for mi in g)
            vt_ps = ps_work.tile([128, gw], F32, tag="work", name="vt_ps")
            base = 0
            for mi in g:
                w = mwidth(mi)
                for c in range(ncw):
                    for kidx in range(len(ktiles)):
                        nc.tensor.matmul(
                            vt_ps[0:cwid, base + c * w : base + (c + 1) * w],
                            msrc(mi, kidx)[:, c * cwid : c * cwid + cwid],
                            av_ap(
                                kidx,
                                ktiles[kidx][0],
                                AV_OFF[l],
                                AV_OFF[l] + w,
                            ),
                            start=(kidx == 0),
                            stop=(kidx == len(ktiles) - 1),
                        )
                base += ncw * w
            vt_sb = sb2.tile([128, gw], DT_BOX, tag="vt_sb", name="vt_sb")
            copy_cast(vt_sb[0:cwid, :], vt_ps[0:cwid, :])
            base = 0
            for mi in g:
                w = mwidth(mi)
                vts[mi] = (vt_sb[:, base : base + ncw * w], w)
                base += ncw * w
        return vts

    def box_h(l, vts):
        """Horizontal pass: stationary = vT chunk, mobile = box band."""
        Wl, Rl = H[l], R[l]
        cwid = min(128, Wl)
        box_ps = {}
        for mi in range(5):
            if l == 0:
                bp = ps_box.tile([Rl, Wl], F32, tag="box", name="bp")
                out_base = 0
            else:
                bp = box_deep[mi]
                out_base = deep_off[l]
            vt, st = vts[mi]
            for (j, lo, hi), off in zip(BH_CH[l], BH_OFF[l]):
                nc.tensor.matmul(
                    bp[0:Rl, out_base + lo : out_base + hi],
                    vt[0:cwid, j * st : j * st + Rl],
                    bh_ap(cwid, off, off + (hi - lo)),
                    start=(j == 0),
                    stop=(j == len(BH_CH[l]) - 1),
                )
            box_ps[mi] = bp
        return box_ps

    def ds_h(l, vts):
        """Downsample horizontal pass off the fused box_v output (the dv
        columns ride along in the x maps' vT chunks at offset Rl)."""
        Wl, Rl, nkn, nkp = H[l], R[l], NK[l + 1], NKP[l + 1]
        rch = Wl // 128
        xnext_ps = ps_work.tile([nkp, 2 * (Wl // 2)], F32, tag="work", name="xn_ps")
        for j in range(2 * Wl // 128):
            img, jr = j // rch, j % rch
            (jj, lo, hi) = DH_CH[l][jr]
            off = DH_OFF[l][jr]
            vt, st = vts[img]
            nc.tensor.matmul(
                xnext_ps[:, img * (Wl // 2) + lo : img * (Wl // 2) + hi],
                vt[0:128, jr * st + Rl : jr * st + Rl + nkp],
                dh_ap(128, off, off + (hi - lo)),
                start=(jr == 0),
                stop=(jr == rch - 1),
            )
        copy_cast(xt[l + 1][:], xnext_ps[0:nkn, :])

    def pointwise(box, Rl, Wl, lvls, clamp=False, pp_eng=None, m2c_eng=None):
        """box: dict mi-> PSUM AP rect [Rl, Wl]; lvls: list of
        (level, part_rows, col_lo, col_hi) for the ttr accumulations.
        zz maps are unscaled x*x, so sig1/sig2 fold the x121 here."""
        pp_eng = pp_eng or nc.gpsimd
        m1, m2, r11, r22, r12 = (box[i] for i in range(5))
        q1 = sb2.tile([Rl, Wl], F32, tag="q1")
        q2 = sb2.tile([Rl, Wl], F32, tag="q2")
        sig1 = sb2.tile([Rl, Wl], F32, tag="sig1")
        sig2 = sb2.tile([Rl, Wl], F32, tag="sig2")
        q12 = sb2.tile([Rl, Wl], F32, tag="q12")
        sig12 = sb2.tile([Rl, Wl], F32, tag="sig12")
        pp = sb2.tile([Rl, Wl], F32, tag="pp")
        rr = sb2.tile([Rl, Wl], F32, tag="rr")
        cs = sb2.tile([Rl, Wl], F32, tag="cs")

        m2c = sb2.tile([Rl, Wl], F32, tag="m2c")
        nc.scalar.activation(q1[:], m1, ActFn.Square)
        nc.scalar.activation(q2[:], m2, ActFn.Square)
        # only one operand of a DVE op may live in PSUM -> stage m2 in SBUF
        nc.scalar.activation(m2c[:], m2, ActFn.Copy)
        nc.vector.scalar_tensor_tensor(
            sig1[:], r11, 121.0, q1[:], AluOp.mult, AluOp.subtract
        )
        nc.vector.scalar_tensor_tensor(
            sig2[:], r22, 121.0, q2[:], AluOp.mult, AluOp.subtract
        )
        nc.vector.tensor_tensor(q12[:], m1, m2c[:], AluOp.mult)
        nc.vector.scalar_tensor_tensor(
            sig12[:], r12, 121.0, q12[:], AluOp.mult, AluOp.subtract
        )
        pp_eng.tensor_tensor(pp[:], sig1[:], sig2[:], AluOp.mult)
        if clamp:
            # keep unused (never-reduced) lanes finite through rsqrt
            nc.vector.tensor_scalar_max(pp[:], pp[:], 1e-20)
        # 1/sqrt(|pp|) in one ACT op (same table set as Square/Copy)
        nc.scalar.activation(rr[:], pp[:], ActFn.Abs_reciprocal_sqrt)
        for (lv, pr, clo, chi) in lvls:
            # C = sig12*r summed along the free axis; tensor_tensor_reduce
            # crashes the device (NRT unrecoverable), stt+accum_out works
            nc.vector.scalar_tensor_tensor(
                cs[0:pr, clo:chi],
                sig12[0:pr, clo:chi],
                1.0,
                rr[0:pr, clo:chi],
                AluOp.mult,
                AluOp.mult,
                accum_out=acc[0:pr, lv : lv + 1],
            )

    def downsample(l):
        """xt[l] -> xt[l+1]: transposed-output vertical stride-2, then dh."""
        Wl, nk, nkn, nkp = H[l], NK[l], NK[l + 1], NKP[l + 1]
        t = xt[l]
        ktiles = KT[l]
        # vertical: stationary = x chunk, mobile = Dv [K, nkp]; vT chunks land
        # in PSUM in pieces to bound bank usage
        nch = 2 * Wl // 128
        npieces = 2 if Wl >= 512 else 1
        chpp = nch // npieces
        vt_sb = sb2.tile([128, nch * nkp], DT_BOX, tag="vt_sb", name="vt_sb")
        for pc in range(npieces):
            vt_ps = ps_work.tile([128, chpp * nkp], F32, tag="work", name="vt_ps")
            for cc in range(chpp):
                c = pc * chpp + cc
                for kidx in range(len(ktiles)):
                    kk, cb = ktiles[kidx]
                    nc.tensor.matmul(
                        vt_ps[:, cc * nkp : (cc + 1) * nkp],
                        t[0:kk, cb + c * 128 : cb + (c + 1) * 128],
                        dv_ap(kidx, kk, DV_OFF[l], DV_OFF[l] + nkp),
                        start=(kidx == 0),
                        stop=(kidx == len(ktiles) - 1),
                    )
            copy_cast(
                vt_sb[:, pc * chpp * nkp : (pc + 1) * chpp * nkp], vt_ps[:]
            )
        # horizontal: stationary = vT chunk [128, nkp], mobile = Dh window
        rch = Wl // 128
        xnext_ps = ps_work.tile([nkp, 2 * (Wl // 2)], F32, tag="work", name="xn_ps")
        for j in range(2 * Wl // 128):
            reg, jr = j // rch, j % rch
            (jj, lo, hi) = DH_CH[l][jr]
            off = DH_OFF[l][jr]
            nc.tensor.matmul(
                xnext_ps[:, reg * (Wl // 2) + lo : reg * (Wl // 2) + hi],
                vt_sb[:, j * nkp : (j + 1) * nkp],
                dh_ap(128, off, off + (hi - lo)),
                start=(jr == 0),
                stop=(jr == rch - 1),
            )
        copy_cast(xt[l + 1][:], xnext_ps[0:nkn, :])
        return xnext_ps

    # ---------------- main schedule ----------------
    # downsample chain first (it is the critical path into the deep levels);
    # level-0 box + pointwise fill the other engines behind it
    xn0 = downsample(0)
    b0v = box_v(0, ((0,), (1,), (2,), (3,), (4,)))
    box0 = box_h(0, b0v)
    # all deep V-passes are emitted before any pointwise: their DVE/ACT/Pool
    # dependencies must not queue behind the long pointwise chains
    b1v = box_v(1, ((0, 1), (2, 3), (4,)), fuse_ds=True, xn_ps=xn0)
    xn1 = ds_h(1, b1v)
    b2v = box_v(2, ((0, 1, 2, 3, 4),), fuse_ds=True, xn_ps=xn1)
    xn2 = ds_h(2, b2v)
    b3v = box_v(3, ((0, 1, 2, 3, 4),), xn_ps=xn2)
    pointwise(
        {i: box0[i][:, :] for i in range(5)},
        128,
        512,
        [(0, 128, 0, 512)],
        pp_eng=nc.gpsimd,
        m2c_eng=nc.gpsimd,
    )

    box_deep = [
        ps_box.tile([128, 448], F32, tag="box", name=f"boxdeep{m}") for m in range(5)
    ]
    box_h(1, b1v)
    pointwise(
        {i: box_deep[i][0:64, 0:256] for i in range(5)},
        64,
        256,
        [(1, 64, 0, 256)],
        pp_eng=nc.vector,
        m2c_eng=nc.scalar,
    )
    box_h(2, b2v)
    pointwise(
        {i: box_deep[i][0:32, 256:384] for i in range(5)},
        32,
        128,
        [(2, 32, 0, 128)],
        pp_eng=nc.vector,
        m2c_eng=nc.scalar,
    )
    box_h(3, b3v)
    pointwise(
        {i: box_deep[i][0:16, 384:448] for i in range(5)},
        16,
        64,
        [(3, 16, 0, 64)],
        pp_eng=nc.vector,
        m2c_eng=nc.scalar,
    )

    nc.sync.dma_start(outp[:], acc[:])


# ----------------------------------------------------------------------------
# public entry point
# ----------------------------------------------------------------------------
_NC_CACHE = {}


def _get_program():
    if "nc" not in _NC_CACHE:
        _NC_CACHE["nc"] = build_program()
    return _NC_CACHE["nc"]


def _core_inputs(img1, img2):
    if "shared" not in _NC_CACHE:
        bhm, dhm = _build_shared_mats()
        _NC_CACHE["shared"] = np.ascontiguousarray(
            np.concatenate([bhm, dhm], axis=1)
        )
        _NC_CACHE["core"] = [
            np.ascontiguousarray(_pack_band_rows(_build_core_mats(i)))
            for i in range(4)
        ]
    maps = []
    for c in range(8):
        b, i = c // 4, c % 4
        maps.append(
            {
                "ximg": _band_slices(img1, img2, b, i),
                "avdm": _NC_CACHE["core"][i],
                "bhdm": _NC_CACHE["shared"],
            }
        )
    return maps


def _finish(results):
    total = 0.0
    for l in range(NLVL):
        s = 0.0
        for c in range(8):
            s += float(np.sum(results[c]["out"][0 : R[l], l].astype(np.float64)))
        mean_c = s / (2.0 * H[l] * H[l])
        total += PYR_W[l] * (2.0 - 2.0 * mean_c)
    return np.float32(total)


def kernel(img1, img2, _run_kwargs=None):
    img1 = np.asarray(img1, np.float32)
    img2 = np.asarray(img2, np.float32)
    nc = _get_program()
    in_maps = _core_inputs(img1, img2)
    res = run_bass_kernel_spmd(nc, in_maps, list(range(8)), **(_run_kwargs or {}))
    out = _finish(res.results)
    if _run_kwargs:
        return out, res
    return out

